# revision 10
# baseline (speedup 1.0000x reference)
"""Trainium2 Bass kernel for DiffKS (differentiable Karplus-Strong string).

Math (per sequence b, time n):
    g = 0.99*l_b[...,0]; p = l_b[...,1]
    b0 = g*(1-p); a1 = g*p
    f0c = f0 - a1/(b0+a1+1e-7)
    z = floor(f0c); zc = z-2; alpha = f0c - zc
    w_j = Lagrange weights (order 5), j=0..5
    block_j = b0*w_j + a1*w_{j-1}, j=0..6           (7 taps)
    taps live at k = c0+j, c0 = zc-1 = z-3 in [36, 96]
    y[n] = x[n] + sum_j block_j[n] * y[n-1-(c0[n]+j)]    (delays 37..103)

Key structure: minimum delay is 37 > 32, so 32-sample chunks are internally
parallel.  Chunk c is computed as 4 accumulating 32x32 PE matmuls against the
previous 4 chunks' outputs, with per-chunk tap matrices built on-chip by a
GPSIMD local_scatter + DVE 32x32 block transpose.  The B=16 batch is sharded
2 sequences per NeuronCore across 8 cores (pure data parallel).

PE constraint: stationary and moving operands must share the same partition
base quadrant, so ALL per-seq data (taps, y ring, x, psum) for seq b lives at
partitions [32b, 32b+32).

Layouts (per core, seqs b=0,1; chunk T=32; NCH = N/32 chunks; NP = N/128):
  natural plane  nat[P, b*128+j]  = q[b, 128*P + j]          [NP, 256]
  S-plane        qS[32b+f, c]     = q[b, 32*c + f]           [64, NCH]
  y ring         ytile[32b+f, 4+c] = y[b, 32*c + f]          [64, 4+NCH]
     (columns 0..3 are zeros = initial state)
Tap matrix for chunk c, source chunk c-q (q=1..4):
  S_q[p, f] = tapval at A[32c+f, 32q-1+f-p]     (lhsT for the PE matmul)
Scatter (groups of G=4 chunks, slot t=c%4): tap (j) of sample (b,c,f) goes to
  Traw[32b+f, 256t + 2*p' (+1)] (u16 pairs), p' = 32*fl + 31 - m,
  v = c0+j-f, fl = v//32 (=q-1), m = v%32 (p = 31-m);
then one DVE 32x32-block transpose gives
  T4[32b+p, 128t + 32(q-1) + f] = S_q^{(c)}[p, f].
"""

import numpy as np

import concourse.bass as bass
import concourse.mybir as mybir
import concourse.bacc as bacc
import concourse.tile as tile
from concourse import bass_utils

F32 = mybir.dt.float32
I32 = mybir.dt.int32
I16 = mybir.dt.int16
U16 = mybir.dt.uint16
AO = mybir.AluOpType
AF = mybir.ActivationFunctionType

B_FULL = 16
N_FULL = 16384
NCORES = 8
B_LOC = 2  # sequences per core
G = 4      # chunks per scatter group

# Lagrange denominators 1/d_j for order 5
INV_D = [-1.0 / 120, 1.0 / 24, -1.0 / 12, 1.0 / 12, -1.0 / 24, 1.0 / 120]


def build_kernel(tc, out_d, f0_d, x_d, lb_d, N):
    nc = tc.nc
    NP = N // 128          # natural-plane columns per seq
    NCH = N // 32          # chunks per seq
    NG = NCH // G          # scatter groups
    assert NP * 128 == N and NP <= 128 and NG * G == NCH

    import contextlib
    ctx = contextlib.ExitStack()
    pp = ctx.enter_context(tc.tile_pool(name="persist", bufs=1))
    traw_pool = ctx.enter_context(tc.tile_pool(name="traw", bufs=3))
    t4_pool = ctx.enter_context(tc.tile_pool(name="t4", bufs=4))
    psum_tr = ctx.enter_context(tc.tile_pool(name="psum_tr", bufs=2, space="PSUM"))
    psum_c = ctx.enter_context(tc.tile_pool(name="psum_c", bufs=4, space="PSUM"))

    with ctx:
        # ---------------- phase 0: load + elementwise tap math ----------------
        nat_f0 = pp.tile([NP, 256], F32)
        nat_x = pp.tile([NP, 256], F32)
        nat_lb = pp.tile([NP, 512], F32)
        for b in range(B_LOC):
            nc.sync.dma_start(
                out=nat_f0[:, b * 128:(b + 1) * 128],
                in_=f0_d[b].rearrange("(p j) -> p j", j=128),
            )
            nc.sync.dma_start(
                out=nat_x[:, b * 128:(b + 1) * 128],
                in_=x_d[b].rearrange("(p j) -> p j", j=128),
            )
            nc.sync.dma_start(
                out=nat_lb[:, b * 256:(b + 1) * 256],
                in_=lb_d[b].rearrange("(p j) s -> p (j s)", j=128),
            )
        # strided views of l_b: even cols = g, odd cols = p
        lb_r = nat_lb[:].rearrange("p (j s) -> p j s", s=2)
        g_ap = lb_r[:, :, 0]  # [NP, 256] stride-2
        p_ap = lb_r[:, :, 1]

        g99 = pp.tile([NP, 256], F32)
        t_gp = pp.tile([NP, 256], F32)   # a1 = 0.99*g*p
        b0t = pp.tile([NP, 256], F32)
        rec = pp.tile([NP, 256], F32)
        f0c = pp.tile([NP, 256], F32)
        zf = pp.tile([NP, 256], F32)
        tmp1 = pp.tile([NP, 256], F32)
        tmp2 = pp.tile([NP, 256], F32)
        itmp = pp.tile([NP, 256], I32)

        V = nc.vector
        V.tensor_scalar(out=g99[:], in0=g_ap, scalar1=0.99, scalar2=None, op0=AO.mult)
        V.tensor_tensor(out=t_gp[:], in0=g99[:], in1=p_ap, op=AO.mult)      # a1
        V.tensor_tensor(out=b0t[:], in0=g99[:], in1=t_gp[:], op=AO.subtract)  # b0
        V.tensor_scalar(out=tmp1[:], in0=g99[:], scalar1=1e-7, scalar2=None, op0=AO.add)
        V.reciprocal(out=rec[:], in_=tmp1[:])
        V.tensor_tensor(out=tmp2[:], in0=t_gp[:], in1=rec[:], op=AO.mult)   # a1/(b0+a1+eps)
        V.tensor_tensor(out=f0c[:], in0=nat_f0[:], in1=tmp2[:], op=AO.subtract)
        # zf = floor(f0c), robust to cast rounding mode
        V.tensor_copy(out=itmp[:], in_=f0c[:])
        V.tensor_copy(out=zf[:], in_=itmp[:])
        V.tensor_tensor(out=tmp1[:], in0=zf[:], in1=f0c[:], op=AO.is_gt)
        V.tensor_tensor(out=zf[:], in0=zf[:], in1=tmp1[:], op=AO.subtract)
        # D = f0c - zf  (alpha = D + 2);  u_m = D + (2 - m), m = 0..5
        D = f0c
        V.tensor_tensor(out=D[:], in0=f0c[:], in1=zf[:], op=AO.subtract)

        u = [pp.tile([NP, 256], F32, name=f"u{m}", tag=f"u{m}") for m in range(6)]
        for m in range(6):
            V.tensor_scalar(out=u[m][:], in0=D[:], scalar1=float(2 - m),
                            scalar2=None, op0=AO.add)
        # prefix[j] = u0*..*u_{j-1}, suffix[j] = u_j*..*u5
        pre = [None] * 6
        suf = [None] * 7
        pre[1] = u[0]
        for j in range(2, 6):
            pre[j] = pp.tile([NP, 256], F32, name=f"pre{j}", tag=f"pre{j}")
            V.tensor_tensor(out=pre[j][:], in0=pre[j - 1][:], in1=u[j - 1][:], op=AO.mult)
        suf[5] = u[5]
        for j in range(4, 0, -1):
            suf[j] = pp.tile([NP, 256], F32, name=f"suf{j}", tag=f"suf{j}")
            V.tensor_tensor(out=suf[j][:], in0=suf[j + 1][:], in1=u[j][:], op=AO.mult)
        w = [pp.tile([NP, 256], F32, name=f"w{j}", tag=f"w{j}") for j in range(6)]
        V.tensor_scalar(out=w[0][:], in0=suf[1][:], scalar1=INV_D[0], scalar2=None, op0=AO.mult)
        for j in range(1, 5):
            V.scalar_tensor_tensor(out=w[j][:], in0=pre[j][:], scalar=INV_D[j],
                                   in1=suf[j + 1][:], op0=AO.mult, op1=AO.mult)
        V.tensor_scalar(out=w[5][:], in0=pre[5][:], scalar1=INV_D[5], scalar2=None, op0=AO.mult)

        # block_j = b0*w_j + a1*w_{j-1}, j=0..6
        blk = [pp.tile([NP, 256], F32, name=f"blk{j}", tag=f"blk{j}") for j in range(7)]
        V.tensor_tensor(out=blk[0][:], in0=b0t[:], in1=w[0][:], op=AO.mult)
        for j in range(1, 6):
            V.tensor_tensor(out=blk[j][:], in0=b0t[:], in1=w[j][:], op=AO.mult)
            V.tensor_tensor(out=tmp1[:], in0=t_gp[:], in1=w[j - 1][:], op=AO.mult)
            V.tensor_tensor(out=blk[j][:], in0=blk[j][:], in1=tmp1[:], op=AO.add)
        V.tensor_tensor(out=blk[6][:], in0=t_gp[:], in1=w[5][:], op=AO.mult)

        # ------------- transposes: natural [NP,128] -> S-layout [64,NCH] -------------
        ident = pp.tile([128, 128], F32)
        nc.gpsimd.memset(ident[:], 1.0)
        nc.gpsimd.affine_select(out=ident[:], in_=ident[:], pattern=[[1, 128]],
                                compare_op=AO.is_equal, fill=0.0, base=0,
                                channel_multiplier=-1)
        # 64x32 "double identity" for the output transposes (rows 32:64 too)
        ident64 = pp.tile([64, 32], F32)
        nc.gpsimd.memset(ident64[:], 1.0)
        nc.gpsimd.affine_select(out=ident64[0:32, :], in_=ident64[0:32, :],
                                pattern=[[1, 32]], compare_op=AO.is_equal,
                                fill=0.0, base=0, channel_multiplier=-1)
        nc.gpsimd.affine_select(out=ident64[32:64, :], in_=ident64[32:64, :],
                                pattern=[[1, 32]], compare_op=AO.is_equal,
                                fill=0.0, base=0, channel_multiplier=-1)

        blkS = pp.tile([64, NCH, 7], F32)
        zfS = pp.tile([64, NCH], F32)
        xS = pp.tile([64, NCH], F32)

        def to_s_plane(src_plane, dst_ap_fn):
            """src_plane: [NP, 256] natural (b-halves).  dst_ap_fn(b, g0) -> [32, NP] AP.

            Transpose-matmul outputs must start at PSUM partition 0, so for
            b=1 we widen the stationary to 64 columns: the wanted rows then
            land at psum partitions [32:64) with the output still based at 0.
            """
            for b in range(B_LOC):
                for g0 in range(4):
                    ps = psum_tr.tile([64, NP], F32, name="ps_tr", tag="ps_tr")
                    if b == 0:
                        src = src_plane[:, 32 * g0: 32 * g0 + 32]
                        nc.tensor.transpose(ps[0:32, :], src, ident[:NP, :NP])
                    else:
                        src = src_plane[:, 96 + 32 * g0: 160 + 32 * g0]
                        nc.tensor.transpose(ps[0:64, :], src, ident[:NP, :NP])
                    V.tensor_copy(out=dst_ap_fn(b, g0), in_=ps[32 * b:32 * b + 32, :])

        for j in range(7):
            to_s_plane(
                blk[j][:],
                lambda b, g0, j=j: blkS[32 * b:32 * b + 32, :, j]
                .rearrange("p (P g) -> p P g", g=4)[:, :, g0],
            )
        to_s_plane(
            zf[:],
            lambda b, g0: zfS[32 * b:32 * b + 32, :]
            .rearrange("p (P g) -> p P g", g=4)[:, :, g0],
        )
        to_s_plane(
            nat_x[:],
            lambda b, g0: xS[32 * b:32 * b + 32, :]
            .rearrange("p (P g) -> p P g", g=4)[:, :, g0],
        )

        # ---------------- scatter index computation ----------------
        # f = partition % 32 ; v_j = (zfS - 3) + j - f ; fl = v_j // 32 ;
        # p' = 32*fl + 31 - (v_j % 32) ; u16 idx = 256*(c%4) + 2*p' (+1)
        fi = pp.tile([64, 1], I32)
        nc.gpsimd.iota(fi[:], pattern=[[1, 1]], base=0, channel_multiplier=1)
        ff = pp.tile([64, 1], F32)
        V.tensor_copy(out=ff[:], in_=fi[:])
        s1 = pp.tile([64, 1], F32)
        s2 = pp.tile([64, 1], F32)
        s3 = pp.tile([64, 1], F32)
        i1 = pp.tile([64, 1], I32)
        V.tensor_scalar(out=s1[:], in0=ff[:], scalar1=1.0 / 32, scalar2=None, op0=AO.mult)
        V.tensor_copy(out=i1[:], in_=s1[:])
        V.tensor_copy(out=s2[:], in_=i1[:])
        V.tensor_tensor(out=s3[:], in0=s2[:], in1=s1[:], op=AO.is_gt)
        V.tensor_tensor(out=s2[:], in0=s2[:], in1=s3[:], op=AO.subtract)   # floor(p/32)
        fmod = pp.tile([64, 1], F32)
        V.scalar_tensor_tensor(out=fmod[:], in0=s2[:], scalar=-32.0, in1=ff[:],
                               op0=AO.mult, op1=AO.add)                    # f = p%32
        sc0 = pp.tile([64, 1], F32)
        V.tensor_scalar(out=sc0[:], in0=fmod[:], scalar1=-1.0, scalar2=-3.0,
                        op0=AO.mult, op1=AO.add)                           # -3 - f

        v0 = pp.tile([64, NCH], F32)
        tA = pp.tile([64, NCH], F32)
        tB = pp.tile([64, NCH], F32)
        tC = pp.tile([64, NCH], F32)
        it16 = pp.tile([64, NCH], I16)
        V.tensor_scalar(out=v0[:], in0=zfS[:], scalar1=sc0[:], scalar2=None, op0=AO.add)
        # fl0 = floor(v0/32) robustly (v0 > 0)
        V.tensor_scalar(out=tA[:], in0=v0[:], scalar1=1.0 / 32, scalar2=None, op0=AO.mult)
        V.tensor_copy(out=it16[:], in_=tA[:])
        V.tensor_copy(out=tB[:], in_=it16[:])
        V.tensor_tensor(out=tC[:], in0=tB[:], in1=tA[:], op=AO.is_gt)
        V.tensor_tensor(out=tB[:], in0=tB[:], in1=tC[:], op=AO.subtract)   # fl0
        # m0 = v0 - 32*fl0 ; t2 = 32*fl0 - m0  (p'_j = t2 + 31 - j + 64*[m0 >= 32-j])
        m0 = tA
        V.scalar_tensor_tensor(out=m0[:], in0=tB[:], scalar=-32.0, in1=v0[:],
                               op0=AO.mult, op1=AO.add)
        t2 = tC
        V.scalar_tensor_tensor(out=t2[:], in0=tB[:], scalar=32.0, in1=m0[:],
                               op0=AO.mult, op1=AO.subtract)

        idxS = pp.tile([64, NCH, 14], I16)
        wj = tB  # reuse
        pj = v0  # reuse
        for j in range(7):
            V.tensor_scalar(out=wj[:], in0=m0[:], scalar1=float(32 - j),
                            scalar2=None, op0=AO.is_ge)
            V.scalar_tensor_tensor(out=pj[:], in0=wj[:], scalar=64.0, in1=t2[:],
                                   op0=AO.mult, op1=AO.add)
            # idx_lo = 2*(pj + 31 - j) = 2*pj + 62-2j
            V.tensor_scalar(out=idxS[:, :, 2 * j], in0=pj[:], scalar1=2.0,
                            scalar2=float(62 - 2 * j), op0=AO.mult, op1=AO.add)
            V.tensor_scalar(out=idxS[:, :, 2 * j + 1], in0=idxS[:, :, 2 * j],
                            scalar1=1, scalar2=None, op0=AO.add)
        # add 256*(c % G) chunk-slot offset to every index
        tmod = pp.tile([64, NCH, 14], I16)
        nc.gpsimd.iota(tmod[:], pattern=[[0, NG], [256, G], [0, 14]], base=0,
                       channel_multiplier=0)
        idx_flat = idxS[:].rearrange("p c j -> p (c j)")
        tmod_flat = tmod[:].rearrange("p c j -> p (c j)")
        V.tensor_tensor(out=idx_flat, in0=idx_flat, in1=tmod_flat, op=AO.add)

        blkS_u16 = blkS[:].bitcast(U16)  # [64, NCH, 14]

        # ---------------- y ring ----------------
        # Split per chunk%4 so Tile's whole-tile dependency tracking yields
        # near-TRUE deps: mm(c, q) waits on evac(c-q)-ish, not evac(c-1).
        # Both seqs share a tile (evac is a single [64,1] op for both).
        ytile = [pp.tile([64, 1 + NCH // 4], F32, name=f"ytile_{g0}",
                         tag=f"ytile_{g0}") for g0 in range(4)]
        for g0 in range(4):
            V.memset(ytile[g0][:, 0:1], 0.0)

        # ---------------- sequential chain over groups/chunks ----------------
        for g in range(NG):
            traw = traw_pool.tile([64, 128 * G], F32, name="traw", tag="traw")
            nc.gpsimd.local_scatter(
                out_ap=traw[:].bitcast(U16),
                data_ap=blkS_u16[:, G * g:G * (g + 1), :].rearrange("p c j -> p (c j)"),
                idxs_ap=idxS[:, G * g:G * (g + 1), :].rearrange("p c j -> p (c j)"),
                channels=64, num_elems=256 * G, num_idxs=14 * G,
            )
            t4 = t4_pool.tile([64, 128 * G], F32, name="t4", tag="t4")
            nc.vector.transpose(out=t4[:], in_=traw[:])
            for t in range(G):
                c = G * g + t
                ps = psum_c.tile([64, 1], F32, name="ps", tag="ps")
                for b in range(B_LOC):
                    bsl = slice(32 * b, 32 * b + 32)
                    for q in range(1, 5):
                        cq = c - q
                        col = 1 + cq // 4
                        nc.tensor.matmul(
                            ps[bsl, 0:1],
                            t4[bsl, 128 * t + 32 * (q - 1):128 * t + 32 * q],
                            ytile[cq % 4][bsl, col:col + 1],
                            start=(q == 1), stop=(q == 4),
                        )
                V.scalar_tensor_tensor(
                    out=ytile[c % 4][:, 1 + c // 4:2 + c // 4],
                    in0=ps[:, 0:1],
                    scalar=1.0, in1=xS[:, c:c + 1],
                    op0=AO.mult, op1=AO.add,
                )

        # ---------------- output transpose + store ----------------
        for b in range(B_LOC):
            bsl = slice(32 * b, 32 * b + 32)
            ynat = pp.tile([NP, 128], F32, name=f"ynat{b}", tag=f"ynat{b}")
            for g0 in range(4):
                src = ytile[g0][bsl, 1:1 + NP]  # [32, NP]
                ps = psum_tr.tile([NP, 32], F32, name="ps_out", tag="ps_tr")
                nc.tensor.transpose(ps[:, :], src, ident64[bsl, :])
                V.tensor_copy(out=ynat[:, 32 * g0:32 * g0 + 32], in_=ps[:, :])
            nc.sync.dma_start(
                out=out_d[b].rearrange("(p j) -> p j", j=128),
                in_=ynat[:],
            )


def build_program(N=N_FULL):
    nc = bacc.Bacc("TRN2", target_bir_lowering=False, debug=False,
                   enable_asserts=False)
    f0_d = nc.dram_tensor("f0", [B_LOC, N], F32, kind="ExternalInput").ap()
    x_d = nc.dram_tensor("x", [B_LOC, N], F32, kind="ExternalInput").ap()
    lb_d = nc.dram_tensor("l_b", [B_LOC, N, 2], F32, kind="ExternalInput").ap()
    out_d = nc.dram_tensor("out", [B_LOC, N], F32, kind="ExternalOutput").ap()
    with tile.TileContext(nc) as tc:
        build_kernel(tc, out_d, f0_d, x_d, lb_d, N)
    nc.compile()
    return nc


_PROGRAM_CACHE = {}


def _get_program(N=N_FULL):
    if N not in _PROGRAM_CACHE:
        _PROGRAM_CACHE[N] = build_program(N)
    return _PROGRAM_CACHE[N]


def kernel(f0, x, l_b, K=108, **kwargs):
    """Full-input entry point: shards batch across 8 cores, returns full output."""
    f0 = np.asarray(f0, dtype=np.float32)
    x = np.asarray(x, dtype=np.float32)
    l_b = np.asarray(l_b, dtype=np.float32)
    B, N = x.shape
    assert B == B_FULL and int(K) == 108
    nc = _get_program(N)
    in_maps = []
    for i in range(NCORES):
        sl = slice(i * B_LOC, (i + 1) * B_LOC)
        in_maps.append({
            "f0": np.ascontiguousarray(f0[sl]),
            "x": np.ascontiguousarray(x[sl]),
            "l_b": np.ascontiguousarray(l_b[sl]),
        })
    res = bass_utils.run_bass_kernel_spmd(nc, in_maps, core_ids=list(range(NCORES)))
    out = np.concatenate([res.results[i]["out"] for i in range(NCORES)], axis=0)
    return out.astype(np.float32)
